# revision 1
# baseline (speedup 1.0000x reference)
"""Bass/Trainium2 kernel for nn_KernelAMController (retrieval_knn).

Math: out(b,:) = -sum_g w_eff(b,g)*adj[tb(b),g,:] / (sum_g w_eff(b,g) + eps)
with w_eff(b,g) = exp(-2*||x_b - p_g||^2) * (counts[tb(b),g] > 0).

Restructured as two matmuls per 512-sample group (data-parallel over B on 8
cores, grid buffers replicated):
  mm1: W^T(g,b) = exp(Pa^T @ Xa)  — augmented split-bf16 matmul gives the full
       exponent -2*||x-p||^2 directly (K=15: hi*hi, hi*lo, lo*hi blocks).
  mm2: Y^T(m,b) = sum_g Ct(g,m) * W^T(g,b) accumulated over 20 g-chunks in
       PSUM, where Ct columns m = d*20+k hold [mask*adj_x | mask*adj_y | mask]
       per time-bin k.
  Selection: one-hot over the 20 bins built from strict > comparisons against
       t_edges (searchsorted-left semantics), applied elementwise to Y^T, then
       reduced over k via a tiny +/-1 block matrix matmul (numerators negated
       there for free). Small PE transposes flip (3,B) -> (B,3) for the final
       per-sample divide.
"""
import numpy as np
import ml_dtypes

import concourse.bass as bass
import concourse.tile as tile
from concourse import mybir, bacc
from concourse.bass_utils import run_bass_kernel_spmd

F32 = mybir.dt.float32
BF16 = mybir.dt.bfloat16
BF16_NP = ml_dtypes.bfloat16

B = 32768
G = 2500
GP = 2560          # padded grid (20 chunks of 128)
NCHUNK = 20
NBINS = 20
NCORES = 8
BC = B // NCORES   # 4096 samples per core
NGRP = 8           # groups per core
BG = BC // NGRP    # 512 samples per group
EPS = 1e-10

_CACHE = {}


def _build_nc():
    nc = bacc.Bacc("TRN2", target_bir_lowering=False)
    x_d = nc.dram_tensor("xstage", [2, BC], F32, kind="ExternalInput")
    on_d = nc.dram_tensor("ones3", [3, BG], BF16, kind="ExternalInput")
    t_d = nc.dram_tensor("trep", [60, BC], F32, kind="ExternalInput")
    pa_d = nc.dram_tensor("pa", [15, GP], BF16, kind="ExternalInput")
    ct_d = nc.dram_tensor("ct", [128, NCHUNK * 64], BF16, kind="ExternalInput")
    ea_d = nc.dram_tensor("ea", [60, 1], F32, kind="ExternalInput")
    eb_d = nc.dram_tensor("eb", [60, 1], F32, kind="ExternalInput")
    bn_d = nc.dram_tensor("bones", [60, 3], BF16, kind="ExternalInput")
    id_d = nc.dram_tensor("ident", [3, 3], F32, kind="ExternalInput")
    o_d = nc.dram_tensor("o", [NGRP, 128, 8], F32, kind="ExternalOutput")

    gt = mybir.AluOpType.is_gt
    with tile.TileContext(nc) as tc:
        with (
            tc.tile_pool(name="consts", bufs=1) as consts,
            tc.tile_pool(name="xin", bufs=2) as xin,
            tc.tile_pool(name="tin", bufs=2) as tin,
            tc.tile_pool(name="xa", bufs=2) as xap,
            tc.tile_pool(name="oh", bufs=2) as oh,
            tc.tile_pool(name="wt", bufs=3) as wtp,
            tc.tile_pool(name="r3", bufs=2) as r3p,
            tc.tile_pool(name="ep", bufs=2) as ep,
            tc.tile_pool(name="pw", bufs=2, space="PSUM") as pwp,
            tc.tile_pool(name="py", bufs=2, space="PSUM") as pyp,
            tc.tile_pool(name="pr", bufs=1, space="PSUM") as prp,
            tc.tile_pool(name="prt", bufs=1, space="PSUM") as prtp,
        ):
            pa_sb = consts.tile([15, GP], BF16)
            nc.sync.dma_start(out=pa_sb[:], in_=pa_d[:])
            ct_sb = consts.tile([128, NCHUNK * 64], BF16)
            nc.sync.dma_start(out=ct_sb[:], in_=ct_d[:])
            ea_sb = consts.tile([60, 1], F32)
            nc.sync.dma_start(out=ea_sb[:], in_=ea_d[:])
            eb_sb = consts.tile([60, 1], F32)
            nc.sync.dma_start(out=eb_sb[:], in_=eb_d[:])
            bn_sb = consts.tile([60, 3], BF16)
            nc.sync.dma_start(out=bn_sb[:], in_=bn_d[:])
            id_sb = consts.tile([3, 3], F32)
            nc.sync.dma_start(out=id_sb[:], in_=id_d[:])
            on_sb = consts.tile([3, BG], BF16)
            nc.sync.dma_start(out=on_sb[:], in_=on_d[:])

            for g in range(NGRP):
                s0 = g * BG
                xf = xin.tile([2, BG], F32)
                nc.sync.dma_start(out=xf[:], in_=x_d[:, s0:s0 + BG])
                tr = tin.tile([60, BG], F32)
                nc.sync.dma_start(out=tr[:], in_=t_d[:, s0:s0 + BG])

                # hi/lo bf16 split of [x0, x1] and [x0^2, x1^2] on partitions 0-1,
                # then DMA-assemble the K=15 moving operand (DMA moves across
                # partitions; compute engines are partition-locked and need
                # 32-aligned bases).
                sq = xap.tile([2, BG], F32, tag="sq")
                nc.vector.tensor_mul(sq[:], xf[:], xf[:])
                xh = xap.tile([2, BG], BF16, tag="xh")
                nc.vector.tensor_copy(xh[:], xf[:])
                xl = xap.tile([2, BG], BF16, tag="xl")
                nc.vector.tensor_sub(xl[:], xf[:], xh[:])
                sqh = xap.tile([2, BG], BF16, tag="sqh")
                nc.vector.tensor_copy(sqh[:], sq[:])
                sql = xap.tile([2, BG], BF16, tag="sql")
                nc.vector.tensor_sub(sql[:], sq[:], sqh[:])
                xa = xap.tile([15, BG], BF16, tag="xa")
                nc.sync.dma_start(out=xa[0:2], in_=xh[:])
                nc.sync.dma_start(out=xa[2:4], in_=sqh[:])
                nc.sync.dma_start(out=xa[5:7], in_=xl[:])
                nc.sync.dma_start(out=xa[7:9], in_=sql[:])
                nc.sync.dma_start(out=xa[10:12], in_=xh[:])
                nc.sync.dma_start(out=xa[12:14], in_=sqh[:])
                xa_c = xa.rearrange("(a b) n -> a b n", b=5)[:, 4, :]
                nc.sync.dma_start(out=xa_c, in_=on_sb[:])

                # one-hot over bins (strict >, searchsorted-left semantics)
                sa = oh.tile([60, BG], F32, tag="sa")
                nc.vector.tensor_scalar(sa[:], tr[:], ea_sb[:], None, gt)
                sb_ = oh.tile([60, BG], F32, tag="sb")
                nc.vector.tensor_scalar(sb_[:], tr[:], eb_sb[:], None, gt)
                o3 = oh.tile([60, BG], BF16, tag="o3")
                nc.vector.tensor_sub(o3[:], sa[:], sb_[:])

                # Software-pipelined by one chunk-pair: pair q+1's mm1s are
                # emitted before pair q's mm2s so the in-order PE queue has
                # independent work while ScalarE computes exp(pair q).
                py = pyp.tile([64, BG], F32)
                pend = None
                for q in range(NCHUNK // 2):
                    pw = pwp.tile([128, 2, BG], F32)
                    for j in (0, 1):
                        c = 2 * q + j
                        nc.tensor.matmul(
                            pw[:, j, :], lhsT=pa_sb[:, c * 128:(c + 1) * 128],
                            rhs=xa[:], start=True, stop=True)
                    wt = wtp.tile([128, 2, BG], BF16)
                    nc.scalar.activation(wt[:], pw[:],
                                         mybir.ActivationFunctionType.Exp)
                    if pend is not None:
                        wp, qp = pend
                        for j in (0, 1):
                            c = 2 * qp + j
                            nc.tensor.matmul(
                                py[:], lhsT=ct_sb[:, c * 64:(c + 1) * 64],
                                rhs=wp[:, j, :], start=(c == 0), stop=False)
                    pend = (wt, q)
                wp, qp = pend
                for j in (0, 1):
                    c = 2 * qp + j
                    nc.tensor.matmul(
                        py[:], lhsT=ct_sb[:, c * 64:(c + 1) * 64],
                        rhs=wp[:, j, :], start=False, stop=(c == NCHUNK - 1))

                r3 = r3p.tile([60, BG], BF16)
                nc.vector.tensor_mul(r3[:], py[0:60, :], o3[:])
                pr = prp.tile([3, BG], F32)
                nc.tensor.matmul(pr[:], lhsT=bn_sb[:], rhs=r3[:], start=True,
                                 stop=True)
                rsb = ep.tile([3, BG], F32, tag="rsb")
                nc.vector.tensor_copy(rsb[:], pr[:])
                prt = prtp.tile([128, 12], F32)
                for s in range(4):
                    nc.tensor.transpose(prt[:, s * 3:(s + 1) * 3],
                                        rsb[:, s * 128:(s + 1) * 128], id_sb[:])
                prt3 = prt.rearrange("p (s c) -> p s c", c=3)
                den = ep.tile([128, 4], F32, tag="den")
                nc.vector.tensor_scalar(den[:], prt3[:, :, 2], EPS, None,
                                        mybir.AluOpType.add)
                rec = ep.tile([128, 4], F32, tag="rec")
                nc.vector.reciprocal(rec[:], den[:])
                ot = ep.tile([128, 8], F32, tag="ot")
                ot2 = ot.rearrange("p (s c) -> p s c", c=2)
                nc.vector.tensor_mul(ot2[:, :, 0], prt3[:, :, 0], rec[:])
                nc.vector.tensor_mul(ot2[:, :, 1], prt3[:, :, 1], rec[:])
                nc.sync.dma_start(out=o_d[g], in_=ot[:])
    nc.compile()
    return nc


def _host_prep(t, x, grid_points, grid_adjoints, t_edges, grid_counts):
    t = np.asarray(t, np.float32).reshape(B)
    x = np.asarray(x, np.float32)
    gp = np.asarray(grid_points, np.float32)
    adj = np.asarray(grid_adjoints, np.float32)
    te = np.asarray(t_edges, np.float32)
    cnt = np.asarray(grid_counts)

    mask = (cnt > 0).astype(np.float32)               # (20, G)
    ct = np.zeros((GP, 64), np.float32)
    ct[:G, 0:20] = (mask * adj[:, :, 0]).T
    ct[:G, 20:40] = (mask * adj[:, :, 1]).T
    ct[:G, 40:60] = mask.T
    ct_dram = np.ascontiguousarray(
        ct.reshape(NCHUNK, 128, 64).transpose(1, 0, 2).reshape(128, NCHUNK * 64)
    ).astype(BF16_NP)

    p5 = np.zeros((5, GP), np.float32)
    p5[0, :G] = 4.0 * gp[:, 0]
    p5[1, :G] = 4.0 * gp[:, 1]
    p5[2, :G] = -2.0
    p5[3, :G] = -2.0
    p5[4, :G] = -2.0 * (gp[:, 0] ** 2 + gp[:, 1] ** 2)
    p5[4, G:] = -1e30
    ph = p5.astype(BF16_NP)
    pl = (p5 - ph.astype(np.float32)).astype(BF16_NP)
    pa = np.concatenate([ph, ph, pl], axis=0)          # (15, GP) bf16

    ea = np.concatenate([[-1.0], te[1:20]]).astype(np.float32)   # (20,)
    eb = te[1:21].astype(np.float32)                              # (20,)
    ea3 = np.tile(ea, 3).reshape(60, 1)
    eb3 = np.tile(eb, 3).reshape(60, 1)

    bones = np.zeros((60, 3), np.float32)
    for d in range(3):
        bones[d * 20:(d + 1) * 20, d] = 1.0 if d == 2 else -1.0
    bones = bones.astype(BF16_NP)
    ident = np.eye(3, dtype=np.float32)

    ones3 = np.zeros((3, BG), np.float32)
    ones3[0] = 1.0
    ones3[2] = 1.0
    ones3 = ones3.astype(BF16_NP)

    in_maps = []
    for i in range(NCORES):
        xs = x[i * BC:(i + 1) * BC]                    # (BC, 2)
        ts = t[i * BC:(i + 1) * BC]                    # (BC,)
        xstage = np.ascontiguousarray(xs.T)            # (2, BC)
        trep = np.ascontiguousarray(np.broadcast_to(ts, (60, BC)))
        in_maps.append({
            "xstage": xstage, "trep": trep, "pa": pa, "ct": ct_dram,
            "ea": ea3, "eb": eb3, "bones": bones, "ident": ident,
            "ones3": ones3,
        })
    return in_maps


def kernel(t, x, grid_points, grid_adjoints, t_edges, grid_counts,
           trace=False, tmpdir=None):
    if "nc" not in _CACHE:
        _CACHE["nc"] = _build_nc()
    nc = _CACHE["nc"]
    in_maps = _host_prep(t, x, grid_points, grid_adjoints, t_edges, grid_counts)
    res = run_bass_kernel_spmd(nc, in_maps, core_ids=list(range(NCORES)),
                               trace=trace, tmpdir=tmpdir)
    _CACHE["last_result"] = res
    out = np.empty((B, 2), np.float32)
    for i in range(NCORES):
        raw = res.results[i]["o"].reshape(NGRP, 128, 4, 2)
        out[i * BC:(i + 1) * BC] = raw.transpose(0, 2, 1, 3).reshape(BC, 2)
    return out



# revision 8
# speedup vs baseline: 5.0438x; 5.0438x over previous
"""Bass/Trainium2 kernel for nn_KernelAMController (retrieval_knn).

Math: out(b,:) = -sum_g w(b,g)*mask[tb,g]*adj[tb(b),g,:] / (sum_g w*mask + eps)
with w(b,g) = exp(-2*||x_b - p_g||^2).

The Gaussian kernel (bandwidth 0.5) is spatially local: grid points beyond
~1.8 units contribute < 3e-4 relative error. Samples are k-d sorted (host)
into 64 leaves of 512 spatially-coherent queries; each leaf only visits the
grid chunks covering its bounding box + margin (~2.5 of 20 chunks). Per
512-sample group on device:
  mm1: exponent(g,b) = Pa^T @ Xa over the leaf's local grid chunks (fp16
       hi/lo split of the quadratic expansion, K=15, built on host).
  exp: ScalarE activation PSUM->SBUF fp16.
  mm2: py[64, 512] += Ct_chunk^T @ W_chunk accumulated in PSUM, columns
       m = d*20+k holding [mask*adj_x | mask*adj_y | mask] per time bin k.
Device streams py out as fp16; the host does the per-sample time-bin
selection and the final -num/(den+eps) divide (O(B) epilogue).

Chunk counts per group slot are data-dependent (computed from the k-d
leaves at call time) and baked into the compiled program; all 8 cores run
the identical program on their own leaf data (SPMD).
"""
import numpy as np
import ml_dtypes

import concourse.bass as bass
import concourse.tile as tile
from concourse import mybir, bacc
from concourse.bass_utils import run_bass_kernel_spmd

F32 = mybir.dt.float32
FP16 = mybir.dt.float16
FP16_NP = ml_dtypes.float16 if hasattr(ml_dtypes, "float16") else np.float16

B = 32768
G = 2500
GSIZE = 50
NBINS = 20
NCORES = 8
BC = B // NCORES   # 4096 samples per core
NGRP = 8           # groups (leaves) per core
BG = BC // NGRP    # 512 samples per leaf
NLEAF = NCORES * NGRP
EPS = 1e-10
MARGIN = 1.8       # neighborhood radius: truncation rel err ~2.5e-4
PAD_EXP = -60000.0  # fp16-representable; exp() -> 0
CB_CLAMP = 20.0    # max per-sample exponent normalization

_CACHE = {}


def _build_nc(caps):
    T = int(sum(caps))
    offs = np.concatenate([[0], np.cumsum(caps)]).astype(int)
    grp_of = np.repeat(np.arange(NGRP), caps)

    nc = bacc.Bacc("TRN2", target_bir_lowering=False)
    xa_d = nc.dram_tensor("xa", [15, BC], FP16, kind="ExternalInput")
    pa_d = nc.dram_tensor("pa", [15, T * 128], FP16, kind="ExternalInput")
    ct_d = nc.dram_tensor("ct", [128, T * 64], FP16, kind="ExternalInput")
    o_d = nc.dram_tensor("o", [64, BC], FP16, kind="ExternalOutput")

    SKEW = 2
    with tile.TileContext(nc) as tc:
        with (
            tc.tile_pool(name="consts", bufs=1) as consts,
            tc.tile_pool(name="wt", bufs=3) as wtp,
            tc.tile_pool(name="pw", bufs=3, space="PSUM") as pwp,
            tc.tile_pool(name="py", bufs=2, space="PSUM") as pyp,
        ):
            pa_sb = consts.tile([15, T * 128], FP16)
            nc.sync.dma_start(out=pa_sb[:], in_=pa_d[:])
            xa_sb = consts.tile([15, BC], FP16)
            nc.sync.dma_start(out=xa_sb[:], in_=xa_d[:])
            ct_sb = consts.tile([128, T * 64], FP16)
            nc.sync.dma_start(out=ct_sb[:], in_=ct_d[:])
            out_sb = consts.tile([64, BC], FP16)

            py_tiles = [None] * NGRP
            pend = []

            def emit_mm2(c, wt):
                g = int(grp_of[c])
                if c == offs[g]:
                    py_tiles[g] = pyp.tile([64, BG], F32, name="py")
                last = c == offs[g + 1] - 1
                nc.tensor.matmul(
                    py_tiles[g][:], lhsT=ct_sb[:, c * 64:(c + 1) * 64],
                    rhs=wt[:], start=(c == offs[g]), stop=last)
                if last:
                    nc.vector.tensor_copy(
                        out_sb[:, g * BG:(g + 1) * BG], py_tiles[g][:])

            for c in range(T):
                g = int(grp_of[c])
                pw = pwp.tile([128, BG], F32)
                nc.tensor.matmul(
                    pw[:], lhsT=pa_sb[:, c * 128:(c + 1) * 128],
                    rhs=xa_sb[:, g * BG:(g + 1) * BG], start=True, stop=True)
                wt = wtp.tile([128, BG], FP16)
                nc.scalar.activation(wt[:], pw[:],
                                     mybir.ActivationFunctionType.Exp)
                pend.append((c, wt))
                if len(pend) > SKEW:
                    emit_mm2(*pend.pop(0))
            for c, wt in pend:
                emit_mm2(c, wt)

            nc.sync.dma_start(out=o_d[:], in_=out_sb[:])
    nc.compile()
    return nc


def _split_leaves(x):
    """Longest-axis k-d median split into 64 leaves of 512 sample indices."""
    leaves = []

    def rec(idx):
        if len(idx) == BG:
            leaves.append(idx)
            return
        xc = np.clip(x[idx], -8.3, 8.3)
        ax = int(np.argmax(xc.max(0) - xc.min(0)))
        order = np.argsort(x[idx, ax], kind="stable")
        h = len(idx) // 2
        rec(idx[order[:h]])
        rec(idx[order[h:]])

    rec(np.arange(x.shape[0]))
    return leaves


def _hi_lo(v):
    hi = v.astype(FP16_NP)
    lo = (v - hi.astype(np.float32)).astype(FP16_NP)
    return hi, lo


def kernel(t, x, grid_points, grid_adjoints, t_edges, grid_counts,
           trace=False, tmpdir=None):
    t = np.asarray(t, np.float32).reshape(B)
    x = np.asarray(x, np.float32)
    gp = np.asarray(grid_points, np.float32)
    adj = np.asarray(grid_adjoints, np.float32)
    te = np.asarray(t_edges, np.float32)
    cnt = np.asarray(grid_counts)

    tb = np.clip(np.searchsorted(te[1:-1], t, side="left"), 0, NBINS - 1)
    lin = gp[:GSIZE, 1]  # linspace(-8, 8, 50): y varies fastest (ij indexing)
    h = float(lin[1] - lin[0])

    # Per-sample exponent normalization c_b = min(2*d^2(nearest grid pt), 20):
    # keeps each sample's max weight near 1 so fp16 W never underflows for
    # spatial outliers. num/den both scale by exp(c_b); the host divide uses
    # eps*exp(c_b) so the result is exactly the reference ratio.
    gnear = np.clip(np.round((x - lin[0]) / h), 0, GSIZE - 1) * h + lin[0]
    cb = np.minimum(2.0 * ((x - gnear) ** 2).sum(1), CB_CLAMP)
    c16 = cb.astype(FP16_NP).astype(np.float32)

    leaves = _split_leaves(x)

    # per-leaf grid neighborhood (index box) and chunk count
    boxes, nchunks = [], []
    for idx in leaves:
        lo = x[idx].min(0) - MARGIN
        hi = x[idx].max(0) + MARGIN
        i0 = int(np.clip(np.searchsorted(lin, lo[0], "left"), 0, GSIZE - 1))
        i1 = int(np.clip(np.searchsorted(lin, hi[0], "right"), i0 + 1, GSIZE))
        j0 = int(np.clip(np.searchsorted(lin, lo[1], "left"), 0, GSIZE - 1))
        j1 = int(np.clip(np.searchsorted(lin, hi[1], "right"), j0 + 1, GSIZE))
        boxes.append((i0, i1, j0, j1))
        nchunks.append(-(-((i1 - i0) * (j1 - j0)) // 128))

    # slot s takes the 8 leaves ranked [8s, 8s+8) by descending chunk count;
    # its capacity is the max in the slot, so all cores share one program.
    order = np.argsort(-np.array(nchunks), kind="stable")
    caps = tuple(int(nchunks[order[8 * s]]) for s in range(NGRP))
    T = sum(caps)
    assign = [[int(order[8 * s + c]) for s in range(NGRP)] for c in range(NCORES)]

    # precompute full-grid quadratic expansion (f32) and ct rows (f32)
    p5 = np.empty((5, G), np.float32)
    p5[0] = 4.0 * gp[:, 0]
    p5[1] = 4.0 * gp[:, 1]
    p5[2] = -2.0
    p5[3] = -2.0
    p5[4] = -2.0 * (gp[:, 0] ** 2 + gp[:, 1] ** 2)
    mask = (cnt > 0).astype(np.float32)                 # (20, G)
    ct_full = np.empty((G, 64), np.float32)
    ct_full[:, 0:20] = (mask * adj[:, :, 0]).T
    ct_full[:, 20:40] = (mask * adj[:, :, 1]).T
    ct_full[:, 40:60] = mask.T
    ct_full[:, 60:64] = 0.0

    in_maps = []
    for c in range(NCORES):
        xa = np.zeros((15, BC), np.float32)
        pa = np.zeros((15, T * 128), FP16_NP)
        pa[4] = PAD_EXP
        pa[9] = 1.0
        ct = np.zeros((T * 128, 64), FP16_NP)
        off = 0
        for s in range(NGRP):
            li = assign[c][s]
            idx = leaves[li]
            i0, i1, j0, j1 = boxes[li]
            ii, jj = np.meshgrid(np.arange(i0, i1), np.arange(j0, j1),
                                 indexing="ij")
            gidx = (ii * GSIZE + jj).reshape(-1)
            n = len(gidx)

            xs = x[idx]
            sl = slice(s * BG, (s + 1) * BG)
            x_hi, x_lo = _hi_lo(xs.T)                   # (2, BG)
            sq_hi, sq_lo = _hi_lo(xs.T.astype(np.float32) ** 2)
            xa[0:2, sl] = x_hi
            xa[2:4, sl] = sq_hi
            xa[4, sl] = 1.0
            xa[5:7, sl] = x_lo
            xa[7:9, sl] = sq_lo
            xa[9, sl] = c16[idx]
            xa[10:12, sl] = x_hi
            xa[12:14, sl] = sq_hi
            xa[14, sl] = 1.0

            p_hi, p_lo = _hi_lo(p5[:, gidx])            # (5, n)
            pa[0:5, 128 * off:128 * off + n] = p_hi
            pa[5:9, 128 * off:128 * off + n] = p_hi[0:4]
            pa[10:15, 128 * off:128 * off + n] = p_lo
            ct[128 * off:128 * off + n] = ct_full[gidx]
            off += caps[s]

        ct_dram = np.ascontiguousarray(
            ct.reshape(T, 128, 64).transpose(1, 0, 2).reshape(128, T * 64))
        in_maps.append({"xa": xa.astype(FP16_NP), "pa": pa, "ct": ct_dram})

    key = ("nc", caps)
    if key not in _CACHE:
        _CACHE[key] = _build_nc(caps)
    nc = _CACHE[key]
    res = run_bass_kernel_spmd(nc, in_maps, core_ids=list(range(NCORES)),
                               trace=trace, tmpdir=tmpdir)
    _CACHE["last_result"] = res

    out = np.empty((B, 2), np.float32)
    jcol = np.arange(BG)
    for c in range(NCORES):
        o = np.asarray(res.results[c]["o"]).astype(np.float32)  # (64, BC)
        for s in range(NGRP):
            idx = leaves[assign[c][s]]
            blk = o[:, s * BG:(s + 1) * BG]
            k = tb[idx]
            den = blk[40 + k, jcol] + EPS * np.exp(c16[idx])
            out[idx, 0] = -blk[k, jcol] / den
            out[idx, 1] = -blk[20 + k, jcol] / den
    return out


# revision 12
# speedup vs baseline: 6.1655x; 1.2224x over previous
"""Bass/Trainium2 kernel for nn_KernelAMController (retrieval_knn).

Math: out(b,:) = -sum_g w(b,g)*mask[tb,g]*adj[tb(b),g,:] / (sum_g w*mask + eps)
with w(b,g) = exp(-2*||x_b - p_g||^2).

The Gaussian kernel (bandwidth 0.5) is spatially local: grid points beyond
~1.8 units contribute < 3e-4 relative error. Samples are k-d sorted (host)
into 64 leaves of 512 spatially-coherent queries; each leaf only visits the
grid chunks covering its bounding box + margin (~2.5 of 20 chunks). Per
512-sample group on device:
  mm1: exponent(g,b) = Pa^T @ Xa over the leaf's local grid chunks (fp16
       hi/lo split of the quadratic expansion, K=15, built on host).
  exp: ScalarE activation PSUM->SBUF fp16.
  mm2: py[64, 512] += Ct_chunk^T @ W_chunk accumulated in PSUM, columns
       m = d*20+k holding [mask*adj_x | mask*adj_y | mask] per time bin k.
Device streams py out as fp16; the host does the per-sample time-bin
selection and the final -num/(den+eps) divide (O(B) epilogue).

Chunk counts per group slot are data-dependent (computed from the k-d
leaves at call time) and baked into the compiled program; all 8 cores run
the identical program on their own leaf data (SPMD).
"""
import numpy as np
import ml_dtypes

import concourse.bass as bass
import concourse.tile as tile
from concourse import mybir, bacc
from concourse.bass_utils import run_bass_kernel_spmd

F32 = mybir.dt.float32
FP16 = mybir.dt.float16
FP16_NP = ml_dtypes.float16 if hasattr(ml_dtypes, "float16") else np.float16

B = 32768
G = 2500
GSIZE = 50
NBINS = 20
NCORES = 8
BC = B // NCORES   # 4096 samples per core
NGRP = 8           # groups (leaves) per core
BG = BC // NGRP    # 512 samples per leaf
NLEAF = NCORES * NGRP
EPS = 1e-10
MARGIN = 1.5       # neighborhood radius: truncation rel err ~1.7e-3
PAD_EXP = -60000.0  # fp16-representable; exp() -> 0
CB_CLAMP = 20.0    # max per-sample exponent normalization

_CACHE = {}


def _build_nc(caps):
    T = int(sum(caps))
    offs = np.concatenate([[0], np.cumsum(caps)]).astype(int)
    grp_of = np.repeat(np.arange(NGRP), caps)

    nc = bacc.Bacc("TRN2", target_bir_lowering=False)
    xa_d = nc.dram_tensor("xa", [15, BC], FP16, kind="ExternalInput")
    pa_d = nc.dram_tensor("pa", [15, T * 128], FP16, kind="ExternalInput")
    ct_d = nc.dram_tensor("ct", [128, T * 64], FP16, kind="ExternalInput")
    o_d = nc.dram_tensor("o", [64, BC], FP16, kind="ExternalOutput")

    SKEW = 3
    with tile.TileContext(nc) as tc:
        with (
            tc.tile_pool(name="consts", bufs=1) as consts,
            tc.tile_pool(name="wt", bufs=4) as wtp,
            tc.tile_pool(name="pw", bufs=4, space="PSUM") as pwp,
            tc.tile_pool(name="py", bufs=2, space="PSUM") as pyp,
        ):
            # pa+xa land first so mm1 starts while ct streams in
            pa_sb = consts.tile([15, T * 128], FP16)
            nc.sync.dma_start(out=pa_sb[:], in_=pa_d[:])
            xa_sb = consts.tile([15, BC], FP16)
            nc.sync.dma_start(out=xa_sb[:], in_=xa_d[:])
            ct_sb = consts.tile([128, T * 64], FP16)
            nc.sync.dma_start(out=ct_sb[:], in_=ct_d[:])
            out_sb = consts.tile([64, BC], FP16)

            py_tiles = [None] * NGRP
            pend = []

            def emit_mm2(c, wt):
                g = int(grp_of[c])
                if c == offs[g]:
                    py_tiles[g] = pyp.tile([64, BG], F32, name="py")
                last = c == offs[g + 1] - 1
                nc.tensor.matmul(
                    py_tiles[g][:], lhsT=ct_sb[:, c * 64:(c + 1) * 64],
                    rhs=wt[:], start=(c == offs[g]), stop=last)
                if last:
                    nc.vector.tensor_copy(
                        out_sb[:, g * BG:(g + 1) * BG], py_tiles[g][:])
                    if g % 2 == 1:  # stream results out per group pair
                        nc.sync.dma_start(
                            out=o_d[:, (g - 1) * BG:(g + 1) * BG],
                            in_=out_sb[:, (g - 1) * BG:(g + 1) * BG])

            for c in range(T):
                g = int(grp_of[c])
                pw = pwp.tile([128, BG], F32)
                nc.tensor.matmul(
                    pw[:], lhsT=pa_sb[:, c * 128:(c + 1) * 128],
                    rhs=xa_sb[:, g * BG:(g + 1) * BG], start=True, stop=True)
                wt = wtp.tile([128, BG], FP16)
                nc.scalar.activation(wt[:], pw[:],
                                     mybir.ActivationFunctionType.Exp)
                pend.append((c, wt))
                if len(pend) > SKEW:
                    emit_mm2(*pend.pop(0))
            for c, wt in pend:
                emit_mm2(c, wt)
    nc.compile()
    return nc


def _split_leaves(x):
    """Longest-axis k-d median split into 64 leaves of 512 sample indices."""
    leaves = []

    def rec(idx):
        if len(idx) == BG:
            leaves.append(idx)
            return
        xc = np.clip(x[idx], -8.3, 8.3)
        ax = int(np.argmax(xc.max(0) - xc.min(0)))
        order = np.argsort(x[idx, ax], kind="stable")
        h = len(idx) // 2
        rec(idx[order[:h]])
        rec(idx[order[h:]])

    rec(np.arange(x.shape[0]))
    return leaves


def _hi_lo(v):
    hi = v.astype(FP16_NP)
    lo = (v - hi.astype(np.float32)).astype(FP16_NP)
    return hi, lo


def kernel(t, x, grid_points, grid_adjoints, t_edges, grid_counts,
           trace=False, tmpdir=None):
    t = np.asarray(t, np.float32).reshape(B)
    x = np.asarray(x, np.float32)
    gp = np.asarray(grid_points, np.float32)
    adj = np.asarray(grid_adjoints, np.float32)
    te = np.asarray(t_edges, np.float32)
    cnt = np.asarray(grid_counts)

    tb = np.clip(np.searchsorted(te[1:-1], t, side="left"), 0, NBINS - 1)
    lin = gp[:GSIZE, 1]  # linspace(-8, 8, 50): y varies fastest (ij indexing)
    h = float(lin[1] - lin[0])

    # Per-sample exponent normalization c_b = min(2*d^2(nearest grid pt), 20):
    # keeps each sample's max weight near 1 so fp16 W never underflows for
    # spatial outliers. num/den both scale by exp(c_b); the host divide uses
    # eps*exp(c_b) so the result is exactly the reference ratio.
    gnear = np.clip(np.round((x - lin[0]) / h), 0, GSIZE - 1) * h + lin[0]
    cb = np.minimum(2.0 * ((x - gnear) ** 2).sum(1), CB_CLAMP)
    c16 = cb.astype(FP16_NP).astype(np.float32)

    leaves = _split_leaves(x)

    # per-leaf grid neighborhood (index box) and chunk count
    boxes, nchunks = [], []
    for idx in leaves:
        lo = x[idx].min(0) - MARGIN
        hi = x[idx].max(0) + MARGIN
        i0 = int(np.clip(np.searchsorted(lin, lo[0], "left"), 0, GSIZE - 1))
        i1 = int(np.clip(np.searchsorted(lin, hi[0], "right"), i0 + 1, GSIZE))
        j0 = int(np.clip(np.searchsorted(lin, lo[1], "left"), 0, GSIZE - 1))
        j1 = int(np.clip(np.searchsorted(lin, hi[1], "right"), j0 + 1, GSIZE))
        boxes.append((i0, i1, j0, j1))
        nchunks.append(-(-((i1 - i0) * (j1 - j0)) // 128))

    # slot s takes the 8 leaves ranked [8s, 8s+8) by descending chunk count;
    # its capacity is the max in the slot, so all cores share one program.
    order = np.argsort(-np.array(nchunks), kind="stable")
    caps = tuple(int(nchunks[order[8 * s]]) for s in range(NGRP))
    T = sum(caps)
    assign = [[int(order[8 * s + c]) for s in range(NGRP)] for c in range(NCORES)]

    # precompute full-grid quadratic expansion (f32) and ct rows (f32)
    p5 = np.empty((5, G), np.float32)
    p5[0] = 4.0 * gp[:, 0]
    p5[1] = 4.0 * gp[:, 1]
    p5[2] = -2.0
    p5[3] = -2.0
    p5[4] = -2.0 * (gp[:, 0] ** 2 + gp[:, 1] ** 2)
    mask = (cnt > 0).astype(np.float32)                 # (20, G)
    ct_full = np.empty((G, 64), np.float32)
    ct_full[:, 0:20] = (mask * adj[:, :, 0]).T
    ct_full[:, 20:40] = (mask * adj[:, :, 1]).T
    ct_full[:, 40:60] = mask.T
    ct_full[:, 60:64] = 0.0

    in_maps = []
    for c in range(NCORES):
        xa = np.zeros((15, BC), np.float32)
        pa = np.zeros((15, T * 128), FP16_NP)
        pa[4] = PAD_EXP
        pa[9] = 1.0
        ct = np.zeros((T * 128, 64), FP16_NP)
        off = 0
        for s in range(NGRP):
            li = assign[c][s]
            idx = leaves[li]
            i0, i1, j0, j1 = boxes[li]
            ii, jj = np.meshgrid(np.arange(i0, i1), np.arange(j0, j1),
                                 indexing="ij")
            gidx = (ii * GSIZE + jj).reshape(-1)
            n = len(gidx)

            xs = x[idx]
            sl = slice(s * BG, (s + 1) * BG)
            x_hi, x_lo = _hi_lo(xs.T)                   # (2, BG)
            sq_hi, sq_lo = _hi_lo(xs.T.astype(np.float32) ** 2)
            xa[0:2, sl] = x_hi
            xa[2:4, sl] = sq_hi
            xa[4, sl] = 1.0
            xa[5:7, sl] = x_lo
            xa[7:9, sl] = sq_lo
            xa[9, sl] = c16[idx]
            xa[10:12, sl] = x_hi
            xa[12:14, sl] = sq_hi
            xa[14, sl] = 1.0

            p_hi, p_lo = _hi_lo(p5[:, gidx])            # (5, n)
            pa[0:5, 128 * off:128 * off + n] = p_hi
            pa[5:9, 128 * off:128 * off + n] = p_hi[0:4]
            pa[10:15, 128 * off:128 * off + n] = p_lo
            ct[128 * off:128 * off + n] = ct_full[gidx]
            off += caps[s]

        ct_dram = np.ascontiguousarray(
            ct.reshape(T, 128, 64).transpose(1, 0, 2).reshape(128, T * 64))
        in_maps.append({"xa": xa.astype(FP16_NP), "pa": pa, "ct": ct_dram})

    key = ("nc", caps)
    if key not in _CACHE:
        _CACHE[key] = _build_nc(caps)
    nc = _CACHE[key]
    res = run_bass_kernel_spmd(nc, in_maps, core_ids=list(range(NCORES)),
                               trace=trace, tmpdir=tmpdir)
    _CACHE["last_result"] = res

    out = np.empty((B, 2), np.float32)
    jcol = np.arange(BG)
    for c in range(NCORES):
        o = np.asarray(res.results[c]["o"]).astype(np.float32)  # (64, BC)
        for s in range(NGRP):
            idx = leaves[assign[c][s]]
            blk = o[:, s * BG:(s + 1) * BG]
            k = tb[idx]
            den = blk[40 + k, jcol] + EPS * np.exp(c16[idx])
            out[idx, 0] = -blk[k, jcol] / den
            out[idx, 1] = -blk[20 + k, jcol] / den
    return out


# revision 14
# speedup vs baseline: 6.3814x; 1.0350x over previous
"""Bass/Trainium2 kernel for nn_KernelAMController (retrieval_knn).

Math: out(b,:) = -sum_g w(b,g)*mask[tb,g]*adj[tb(b),g,:] / (sum_g w*mask + eps)
with w(b,g) = exp(-2*||x_b - p_g||^2).

The Gaussian kernel (bandwidth 0.5) is spatially local: grid points beyond
~1.8 units contribute < 3e-4 relative error. Samples are k-d sorted (host)
into 64 leaves of 512 spatially-coherent queries; each leaf only visits the
grid chunks covering its bounding box + margin (~2.5 of 20 chunks). Per
512-sample group on device:
  mm1: exponent(g,b) = Pa^T @ Xa over the leaf's local grid chunks (fp16
       hi/lo split of the quadratic expansion, K=15, built on host).
  exp: ScalarE activation PSUM->SBUF fp16.
  mm2: py[64, 512] += Ct_chunk^T @ W_chunk accumulated in PSUM, columns
       m = d*20+k holding [mask*adj_x | mask*adj_y | mask] per time bin k.
Device streams py out as fp16; the host does the per-sample time-bin
selection and the final -num/(den+eps) divide (O(B) epilogue).

Chunk counts per group slot are data-dependent (computed from the k-d
leaves at call time) and baked into the compiled program; all 8 cores run
the identical program on their own leaf data (SPMD).
"""
import numpy as np
import ml_dtypes

import concourse.bass as bass
import concourse.tile as tile
from concourse import mybir, bacc
from concourse.bass_utils import run_bass_kernel_spmd

F32 = mybir.dt.float32
FP16 = mybir.dt.float16
FP16_NP = ml_dtypes.float16 if hasattr(ml_dtypes, "float16") else np.float16

B = 32768
G = 2500
GSIZE = 50
NBINS = 20
NCORES = 8
BC = B // NCORES   # 4096 samples per core
NGRP = 8           # groups (leaves) per core
BG = BC // NGRP    # 512 samples per leaf
NLEAF = NCORES * NGRP
EPS = 1e-10
MARGIN = 1.5       # neighborhood radius: truncation rel err ~1.7e-3
PAD_EXP = -60000.0  # fp16-representable; exp() -> 0
CB_CLAMP = 20.0    # max per-sample exponent normalization

_CACHE = {}


def _build_nc(caps):
    T = int(sum(caps))
    offs = np.concatenate([[0], np.cumsum(caps)]).astype(int)
    grp_of = np.repeat(np.arange(NGRP), caps)

    nc = bacc.Bacc("TRN2", target_bir_lowering=False)
    xa_d = nc.dram_tensor("xa", [15, BC], FP16, kind="ExternalInput")
    pa_d = nc.dram_tensor("pa", [15, T * 128], FP16, kind="ExternalInput")
    ct_d = nc.dram_tensor("ct", [128, T * 64], FP16, kind="ExternalInput")
    o_d = nc.dram_tensor("o", [64, BC], FP16, kind="ExternalOutput")

    NPAIR = (T + 1) // 2
    with tile.TileContext(nc) as tc:
        with (
            tc.tile_pool(name="consts", bufs=1) as consts,
            tc.tile_pool(name="wt", bufs=3) as wtp,
            tc.tile_pool(name="pw", bufs=3, space="PSUM") as pwp,
            tc.tile_pool(name="py", bufs=2, space="PSUM") as pyp,
        ):
            # input DMAs triggered on three different engines in parallel
            pa_sb = consts.tile([15, T * 128], FP16)
            nc.sync.dma_start(out=pa_sb[:], in_=pa_d[:])
            xa_sb = consts.tile([15, BC], FP16)
            nc.gpsimd.dma_start(out=xa_sb[:], in_=xa_d[:])
            ct_sb = consts.tile([128, T * 64], FP16)
            nc.scalar.dma_start(out=ct_sb[:], in_=ct_d[:])
            out_sb = consts.tile([64, BC], FP16)

            py_tiles = [None] * NGRP
            pend = []

            def emit_mm2(c, wt):
                g = int(grp_of[c])
                if c == offs[g]:
                    py_tiles[g] = pyp.tile([64, BG], F32, name="py")
                last = c == offs[g + 1] - 1
                nc.tensor.matmul(
                    py_tiles[g][:], lhsT=ct_sb[:, c * 64:(c + 1) * 64],
                    rhs=wt[:], start=(c == offs[g]), stop=last)
                if last:
                    nc.vector.tensor_copy(
                        out_sb[:, g * BG:(g + 1) * BG], py_tiles[g][:])
                    if g % 2 == 1:  # stream results out per group pair
                        nc.sync.dma_start(
                            out=o_d[:, (g - 1) * BG:(g + 1) * BG],
                            in_=out_sb[:, (g - 1) * BG:(g + 1) * BG])

            for q in range(NPAIR):
                w = min(2, T - 2 * q)
                pw = pwp.tile([128, 2, BG], F32, name="pw")
                for j in range(w):
                    c = 2 * q + j
                    g = int(grp_of[c])
                    nc.tensor.matmul(
                        pw[:, j, :], lhsT=pa_sb[:, c * 128:(c + 1) * 128],
                        rhs=xa_sb[:, g * BG:(g + 1) * BG],
                        start=True, stop=True)
                wt = wtp.tile([128, 2, BG], FP16, name="wt")
                nc.scalar.activation(wt[:, 0:w, :], pw[:, 0:w, :],
                                     mybir.ActivationFunctionType.Exp)
                pend.append((q, w, wt))
                if len(pend) > 2:
                    qp, wp_, wtp_ = pend.pop(0)
                    for j in range(wp_):
                        emit_mm2(2 * qp + j, wtp_[:, j, :])
            for qp, wp_, wtp_ in pend:
                for j in range(wp_):
                    emit_mm2(2 * qp + j, wtp_[:, j, :])
    nc.compile()
    return nc


def _split_leaves(x):
    """Longest-axis k-d median split into 64 leaves of 512 sample indices."""
    leaves = []

    def rec(idx):
        if len(idx) == BG:
            leaves.append(idx)
            return
        xc = np.clip(x[idx], -8.3, 8.3)
        ax = int(np.argmax(xc.max(0) - xc.min(0)))
        order = np.argsort(x[idx, ax], kind="stable")
        h = len(idx) // 2
        rec(idx[order[:h]])
        rec(idx[order[h:]])

    rec(np.arange(x.shape[0]))
    return leaves


def _hi_lo(v):
    hi = v.astype(FP16_NP)
    lo = (v - hi.astype(np.float32)).astype(FP16_NP)
    return hi, lo


def kernel(t, x, grid_points, grid_adjoints, t_edges, grid_counts,
           trace=False, tmpdir=None):
    t = np.asarray(t, np.float32).reshape(B)
    x = np.asarray(x, np.float32)
    gp = np.asarray(grid_points, np.float32)
    adj = np.asarray(grid_adjoints, np.float32)
    te = np.asarray(t_edges, np.float32)
    cnt = np.asarray(grid_counts)

    tb = np.clip(np.searchsorted(te[1:-1], t, side="left"), 0, NBINS - 1)
    lin = gp[:GSIZE, 1]  # linspace(-8, 8, 50): y varies fastest (ij indexing)
    h = float(lin[1] - lin[0])

    # Per-sample exponent normalization c_b = min(2*d^2(nearest grid pt), 20):
    # keeps each sample's max weight near 1 so fp16 W never underflows for
    # spatial outliers. num/den both scale by exp(c_b); the host divide uses
    # eps*exp(c_b) so the result is exactly the reference ratio.
    gnear = np.clip(np.round((x - lin[0]) / h), 0, GSIZE - 1) * h + lin[0]
    cb = np.minimum(2.0 * ((x - gnear) ** 2).sum(1), CB_CLAMP)
    c16 = cb.astype(FP16_NP).astype(np.float32)

    leaves = _split_leaves(x)

    # per-leaf grid neighborhood (index box) and chunk count
    boxes, nchunks = [], []
    for idx in leaves:
        lo = x[idx].min(0) - MARGIN
        hi = x[idx].max(0) + MARGIN
        i0 = int(np.clip(np.searchsorted(lin, lo[0], "left"), 0, GSIZE - 1))
        i1 = int(np.clip(np.searchsorted(lin, hi[0], "right"), i0 + 1, GSIZE))
        j0 = int(np.clip(np.searchsorted(lin, lo[1], "left"), 0, GSIZE - 1))
        j1 = int(np.clip(np.searchsorted(lin, hi[1], "right"), j0 + 1, GSIZE))
        boxes.append((i0, i1, j0, j1))
        nchunks.append(-(-((i1 - i0) * (j1 - j0)) // 128))

    # slot s takes the 8 leaves ranked [8s, 8s+8) by descending chunk count;
    # its capacity is the max in the slot, so all cores share one program.
    order = np.argsort(-np.array(nchunks), kind="stable")
    caps = tuple(int(nchunks[order[8 * s]]) for s in range(NGRP))
    T = sum(caps)
    assign = [[int(order[8 * s + c]) for s in range(NGRP)] for c in range(NCORES)]

    # precompute full-grid quadratic expansion (f32) and ct rows (f32)
    p5 = np.empty((5, G), np.float32)
    p5[0] = 4.0 * gp[:, 0]
    p5[1] = 4.0 * gp[:, 1]
    p5[2] = -2.0
    p5[3] = -2.0
    p5[4] = -2.0 * (gp[:, 0] ** 2 + gp[:, 1] ** 2)
    mask = (cnt > 0).astype(np.float32)                 # (20, G)
    ct_full = np.empty((G, 64), np.float32)
    ct_full[:, 0:20] = (mask * adj[:, :, 0]).T
    ct_full[:, 20:40] = (mask * adj[:, :, 1]).T
    ct_full[:, 40:60] = mask.T
    ct_full[:, 60:64] = 0.0

    in_maps = []
    for c in range(NCORES):
        xa = np.zeros((15, BC), np.float32)
        pa = np.zeros((15, T * 128), FP16_NP)
        pa[4] = PAD_EXP
        pa[9] = 1.0
        ct = np.zeros((T * 128, 64), FP16_NP)
        off = 0
        for s in range(NGRP):
            li = assign[c][s]
            idx = leaves[li]
            i0, i1, j0, j1 = boxes[li]
            ii, jj = np.meshgrid(np.arange(i0, i1), np.arange(j0, j1),
                                 indexing="ij")
            gidx = (ii * GSIZE + jj).reshape(-1)
            n = len(gidx)

            xs = x[idx]
            sl = slice(s * BG, (s + 1) * BG)
            x_hi, x_lo = _hi_lo(xs.T)                   # (2, BG)
            sq_hi, sq_lo = _hi_lo(xs.T.astype(np.float32) ** 2)
            xa[0:2, sl] = x_hi
            xa[2:4, sl] = sq_hi
            xa[4, sl] = 1.0
            xa[5:7, sl] = x_lo
            xa[7:9, sl] = sq_lo
            xa[9, sl] = c16[idx]
            xa[10:12, sl] = x_hi
            xa[12:14, sl] = sq_hi
            xa[14, sl] = 1.0

            p_hi, p_lo = _hi_lo(p5[:, gidx])            # (5, n)
            pa[0:5, 128 * off:128 * off + n] = p_hi
            pa[5:9, 128 * off:128 * off + n] = p_hi[0:4]
            pa[10:15, 128 * off:128 * off + n] = p_lo
            ct[128 * off:128 * off + n] = ct_full[gidx]
            off += caps[s]

        ct_dram = np.ascontiguousarray(
            ct.reshape(T, 128, 64).transpose(1, 0, 2).reshape(128, T * 64))
        in_maps.append({"xa": xa.astype(FP16_NP), "pa": pa, "ct": ct_dram})

    key = ("nc", caps)
    if key not in _CACHE:
        _CACHE[key] = _build_nc(caps)
    nc = _CACHE[key]
    res = run_bass_kernel_spmd(nc, in_maps, core_ids=list(range(NCORES)),
                               trace=trace, tmpdir=tmpdir)
    _CACHE["last_result"] = res

    out = np.empty((B, 2), np.float32)
    jcol = np.arange(BG)
    for c in range(NCORES):
        o = np.asarray(res.results[c]["o"]).astype(np.float32)  # (64, BC)
        for s in range(NGRP):
            idx = leaves[assign[c][s]]
            blk = o[:, s * BG:(s + 1) * BG]
            k = tb[idx]
            den = blk[40 + k, jcol] + EPS * np.exp(c16[idx])
            out[idx, 0] = -blk[k, jcol] / den
            out[idx, 1] = -blk[20 + k, jcol] / den
    return out


# revision 16
# speedup vs baseline: 7.1041x; 1.1132x over previous
"""Bass/Trainium2 kernel for nn_KernelAMController (retrieval_knn).

Math: out(b,:) = -sum_g w(b,g)*mask[tb,g]*adj[tb(b),g,:] / (sum_g w*mask + eps)
with w(b,g) = exp(-2*||x_b - p_g||^2).

The Gaussian kernel (bandwidth 0.5) is spatially local: grid points beyond
~1.8 units contribute < 3e-4 relative error. Samples are k-d sorted (host)
into 64 leaves of 512 spatially-coherent queries; each leaf only visits the
grid chunks covering its bounding box + margin (~2.5 of 20 chunks). Per
512-sample group on device:
  mm1: exponent(g,b) = Pa^T @ Xa over the leaf's local grid chunks (fp16
       hi/lo split of the quadratic expansion, K=15, built on host).
  exp: ScalarE activation PSUM->SBUF fp16.
  mm2: py[64, 512] += Ct_chunk^T @ W_chunk accumulated in PSUM, columns
       m = d*20+k holding [mask*adj_x | mask*adj_y | mask] per time bin k.
Device streams py out as fp16; the host does the per-sample time-bin
selection and the final -num/(den+eps) divide (O(B) epilogue).

Chunk counts per group slot are data-dependent (computed from the k-d
leaves at call time) and baked into the compiled program; all 8 cores run
the identical program on their own leaf data (SPMD).
"""
import numpy as np
import ml_dtypes

import concourse.bass as bass
import concourse.tile as tile
from concourse import mybir, bacc
from concourse.bass_utils import run_bass_kernel_spmd

F32 = mybir.dt.float32
FP16 = mybir.dt.float16
FP16_NP = ml_dtypes.float16 if hasattr(ml_dtypes, "float16") else np.float16

B = 32768
G = 2500
GSIZE = 50
NBINS = 20
NCORES = 8
BC = B // NCORES   # 4096 samples per core
NGRP = 8           # groups (leaves) per core
BG = BC // NGRP    # 512 samples per leaf
NLEAF = NCORES * NGRP
EPS = 1e-10
MARGIN = 1.4       # neighborhood radius: truncation rel err ~4e-3
PAD_EXP = -60000.0  # fp16-representable; exp() -> 0
CB_CLAMP = 20.0    # max per-sample exponent normalization

_CACHE = {}


def _build_nc(caps):
    T = int(sum(caps))
    offs = np.concatenate([[0], np.cumsum(caps)]).astype(int)
    grp_of = np.repeat(np.arange(NGRP), caps)

    nc = bacc.Bacc("TRN2", target_bir_lowering=False)
    xa_d = nc.dram_tensor("xa", [15, BC], FP16, kind="ExternalInput")
    pa_d = nc.dram_tensor("pa", [15, T * 128], FP16, kind="ExternalInput")
    ct_d = nc.dram_tensor("ct", [128, T * 64], FP16, kind="ExternalInput")
    o_d = nc.dram_tensor("o", [64, BC], FP16, kind="ExternalOutput")

    NPAIR = (T + 1) // 2
    with tile.TileContext(nc) as tc:
        with (
            tc.tile_pool(name="consts", bufs=1) as consts,
            tc.tile_pool(name="wt", bufs=3) as wtp,
            tc.tile_pool(name="pw", bufs=3, space="PSUM") as pwp,
            tc.tile_pool(name="py", bufs=2, space="PSUM") as pyp,
        ):
            # Input DMAs on three engines in parallel, head pieces first so
            # the first matmuls start while the bulk still streams in.
            HD = min(6, T)          # ct head: first 6 chunks
            pa_sb = consts.tile([15, T * 128], FP16)
            nc.sync.dma_start(out=pa_sb[:], in_=pa_d[:])
            xa_sb = consts.tile([15, BC], FP16)
            nc.gpsimd.dma_start(out=xa_sb[:, 0:2 * BG], in_=xa_d[:, 0:2 * BG])
            ct_sb = consts.tile([128, T * 64], FP16)
            nc.scalar.dma_start(out=ct_sb[:, 0:HD * 64], in_=ct_d[:, 0:HD * 64])
            nc.gpsimd.dma_start(out=xa_sb[:, 2 * BG:], in_=xa_d[:, 2 * BG:])
            if T > HD:
                nc.scalar.dma_start(out=ct_sb[:, HD * 64:], in_=ct_d[:, HD * 64:])
            out_sb = consts.tile([64, BC], FP16)

            py_tiles = [None] * NGRP
            pend = []

            def emit_mm2(c, wt):
                g = int(grp_of[c])
                if c == offs[g]:
                    py_tiles[g] = pyp.tile([64, BG], F32, name="py")
                last = c == offs[g + 1] - 1
                nc.tensor.matmul(
                    py_tiles[g][:], lhsT=ct_sb[:, c * 64:(c + 1) * 64],
                    rhs=wt[:], start=(c == offs[g]), stop=last)
                if last:
                    nc.vector.tensor_copy(
                        out_sb[:, g * BG:(g + 1) * BG], py_tiles[g][:])
                    if g % 2 == 1:  # stream results out per group pair
                        nc.sync.dma_start(
                            out=o_d[:, (g - 1) * BG:(g + 1) * BG],
                            in_=out_sb[:, (g - 1) * BG:(g + 1) * BG])

            for q in range(NPAIR):
                w = min(2, T - 2 * q)
                pw = pwp.tile([128, 2, BG], F32, name="pw")
                for j in range(w):
                    c = 2 * q + j
                    g = int(grp_of[c])
                    nc.tensor.matmul(
                        pw[:, j, :], lhsT=pa_sb[:, c * 128:(c + 1) * 128],
                        rhs=xa_sb[:, g * BG:(g + 1) * BG],
                        start=True, stop=True)
                wt = wtp.tile([128, 2, BG], FP16, name="wt")
                nc.scalar.activation(wt[:, 0:w, :], pw[:, 0:w, :],
                                     mybir.ActivationFunctionType.Exp)
                pend.append((q, w, wt))
                if len(pend) > 2:
                    qp, wp_, wtp_ = pend.pop(0)
                    for j in range(wp_):
                        emit_mm2(2 * qp + j, wtp_[:, j, :])
            for qp, wp_, wtp_ in pend:
                for j in range(wp_):
                    emit_mm2(2 * qp + j, wtp_[:, j, :])
    nc.compile()
    return nc


def _split_leaves(x):
    """Longest-axis k-d median split into 64 leaves of 512 sample indices."""
    leaves = []

    def rec(idx):
        if len(idx) == BG:
            leaves.append(idx)
            return
        xc = np.clip(x[idx], -8.3, 8.3)
        ax = int(np.argmax(xc.max(0) - xc.min(0)))
        order = np.argsort(x[idx, ax], kind="stable")
        h = len(idx) // 2
        rec(idx[order[:h]])
        rec(idx[order[h:]])

    rec(np.arange(x.shape[0]))
    return leaves


def _hi_lo(v):
    hi = v.astype(FP16_NP)
    lo = (v - hi.astype(np.float32)).astype(FP16_NP)
    return hi, lo


def kernel(t, x, grid_points, grid_adjoints, t_edges, grid_counts,
           trace=False, tmpdir=None):
    t = np.asarray(t, np.float32).reshape(B)
    x = np.asarray(x, np.float32)
    gp = np.asarray(grid_points, np.float32)
    adj = np.asarray(grid_adjoints, np.float32)
    te = np.asarray(t_edges, np.float32)
    cnt = np.asarray(grid_counts)

    tb = np.clip(np.searchsorted(te[1:-1], t, side="left"), 0, NBINS - 1)
    lin = gp[:GSIZE, 1]  # linspace(-8, 8, 50): y varies fastest (ij indexing)
    h = float(lin[1] - lin[0])

    # Per-sample exponent normalization c_b = min(2*d^2(nearest grid pt), 20):
    # keeps each sample's max weight near 1 so fp16 W never underflows for
    # spatial outliers. num/den both scale by exp(c_b); the host divide uses
    # eps*exp(c_b) so the result is exactly the reference ratio.
    gnear = np.clip(np.round((x - lin[0]) / h), 0, GSIZE - 1) * h + lin[0]
    cb = np.minimum(2.0 * ((x - gnear) ** 2).sum(1), CB_CLAMP)
    c16 = cb.astype(FP16_NP).astype(np.float32)

    leaves = _split_leaves(x)

    # per-leaf grid neighborhood (index box) and chunk count
    boxes, nchunks = [], []
    for idx in leaves:
        lo = x[idx].min(0) - MARGIN
        hi = x[idx].max(0) + MARGIN
        i0 = int(np.clip(np.searchsorted(lin, lo[0], "left"), 0, GSIZE - 1))
        i1 = int(np.clip(np.searchsorted(lin, hi[0], "right"), i0 + 1, GSIZE))
        j0 = int(np.clip(np.searchsorted(lin, lo[1], "left"), 0, GSIZE - 1))
        j1 = int(np.clip(np.searchsorted(lin, hi[1], "right"), j0 + 1, GSIZE))
        boxes.append((i0, i1, j0, j1))
        nchunks.append(-(-((i1 - i0) * (j1 - j0)) // 128))

    # slot s takes the 8 leaves ranked [8s, 8s+8) by descending chunk count;
    # its capacity is the max in the slot, so all cores share one program.
    order = np.argsort(-np.array(nchunks), kind="stable")
    caps = tuple(int(nchunks[order[8 * s]]) for s in range(NGRP))
    T = sum(caps)
    assign = [[int(order[8 * s + c]) for s in range(NGRP)] for c in range(NCORES)]

    # precompute full-grid quadratic expansion (f32) and ct rows (f32)
    p5 = np.empty((5, G), np.float32)
    p5[0] = 4.0 * gp[:, 0]
    p5[1] = 4.0 * gp[:, 1]
    p5[2] = -2.0
    p5[3] = -2.0
    p5[4] = -2.0 * (gp[:, 0] ** 2 + gp[:, 1] ** 2)
    mask = (cnt > 0).astype(np.float32)                 # (20, G)
    ct_full = np.empty((G, 64), np.float32)
    ct_full[:, 0:20] = (mask * adj[:, :, 0]).T
    ct_full[:, 20:40] = (mask * adj[:, :, 1]).T
    ct_full[:, 40:60] = mask.T
    ct_full[:, 60:64] = 0.0

    in_maps = []
    for c in range(NCORES):
        xa = np.zeros((15, BC), np.float32)
        pa = np.zeros((15, T * 128), FP16_NP)
        pa[4] = PAD_EXP
        pa[9] = 1.0
        ct = np.zeros((T * 128, 64), FP16_NP)
        off = 0
        for s in range(NGRP):
            li = assign[c][s]
            idx = leaves[li]
            i0, i1, j0, j1 = boxes[li]
            ii, jj = np.meshgrid(np.arange(i0, i1), np.arange(j0, j1),
                                 indexing="ij")
            gidx = (ii * GSIZE + jj).reshape(-1)
            n = len(gidx)

            xs = x[idx]
            sl = slice(s * BG, (s + 1) * BG)
            x_hi, x_lo = _hi_lo(xs.T)                   # (2, BG)
            sq_hi, sq_lo = _hi_lo(xs.T.astype(np.float32) ** 2)
            xa[0:2, sl] = x_hi
            xa[2:4, sl] = sq_hi
            xa[4, sl] = 1.0
            xa[5:7, sl] = x_lo
            xa[7:9, sl] = sq_lo
            xa[9, sl] = c16[idx]
            xa[10:12, sl] = x_hi
            xa[12:14, sl] = sq_hi
            xa[14, sl] = 1.0

            p_hi, p_lo = _hi_lo(p5[:, gidx])            # (5, n)
            pa[0:5, 128 * off:128 * off + n] = p_hi
            pa[5:9, 128 * off:128 * off + n] = p_hi[0:4]
            pa[10:15, 128 * off:128 * off + n] = p_lo
            ct[128 * off:128 * off + n] = ct_full[gidx]
            off += caps[s]

        ct_dram = np.ascontiguousarray(
            ct.reshape(T, 128, 64).transpose(1, 0, 2).reshape(128, T * 64))
        in_maps.append({"xa": xa.astype(FP16_NP), "pa": pa, "ct": ct_dram})

    key = ("nc", caps)
    if key not in _CACHE:
        _CACHE[key] = _build_nc(caps)
    nc = _CACHE[key]
    res = run_bass_kernel_spmd(nc, in_maps, core_ids=list(range(NCORES)),
                               trace=trace, tmpdir=tmpdir)
    _CACHE["last_result"] = res

    out = np.empty((B, 2), np.float32)
    jcol = np.arange(BG)
    for c in range(NCORES):
        o = np.asarray(res.results[c]["o"]).astype(np.float32)  # (64, BC)
        for s in range(NGRP):
            idx = leaves[assign[c][s]]
            blk = o[:, s * BG:(s + 1) * BG]
            k = tb[idx]
            den = blk[40 + k, jcol] + EPS * np.exp(c16[idx])
            out[idx, 0] = -blk[k, jcol] / den
            out[idx, 1] = -blk[20 + k, jcol] / den
    return out
